# revision 58
# baseline (speedup 1.0000x reference)
"""MoE grouped w8a8 block-quant GEMM + gated combine for 8 Trainium2 cores.

Sharding (expert-parallel, per the hint): core c owns experts [4c,4c+4),
their routed rows [c*16384,(c+1)*16384) (uniform token_count=4096), and
tokens [c*2048,(c+1)*2048). Routed rows are dispatched to their owning
core on the host (the all-to-all dispatch step): x rows are packed
pre-transposed into a K-on-partitions layout, so the device reads them
with fat sequential DMAs. All arithmetic (gate normalization/masking,
scale products, dequant, GEMM, combine) runs on device.

Device pipeline per core:
  Phase A (once): normalize+mask gates and multiply by x-scales in a
    q-major [128, 512] layout (partition q*4+k = kb k, rows q*512..) ->
    xsg table written to DRAM; a duplicate mini phase-A computes chunk
    0's xsg directly at the row-tile partitions (no DRAM roundtrip).
  Phase B: dequant int8 weights -> bf16 on the scalar engine, one
    expert ahead of use, interleaved into the loop.
  Main loop, software-pipelined (S-stage runs LA=2 chunks ahead of the
  GEMM stage), per chunk of R=1024 routed rows (= 128 tokens):
    1. Sequential DMA pulls the pre-dispatched x chunk [128, 2, 2048]
       int8; 4 small DMAs stage this chunk's xsg rows at partitions
       32r..32r+3.
    2. Four concurrent row-tiled K=4 matmuls (tile_position=(32r,0))
       broadcast xsg to 128 partitions (S[p] = xsg[kb(p)]); the scalar
       engine evicts the four 1-bank PSUM tiles to a bf16 S tile.
    3. Two DVE ops dequantize: xdq = int8 * S (bf16), gates folded in.
    4. 32 back-to-back matmuls accumulate the gated expert GEMM into
       [128 tokens, 512] PSUM (top-k combine == PSUM accumulation).
    5. DVE adds shared_output, writes bf16; DMA out.
"""

import numpy as np
import ml_dtypes

T, TOPK, K, N, E, B = 16384, 8, 512, 512, 32, 128
ROWS = T * TOPK
NCORES = 8
EL = E // NCORES            # experts per core
RPC = ROWS // NCORES        # routed rows per core
TPC = T // NCORES           # tokens per core
R = 1024                    # rows per chunk
NCH = RPC // R              # chunks per core

_cache = {}


def _build(n_chunks=NCH):
    from contextlib import ExitStack
    import concourse.bass as bass
    import concourse.bacc as bacc
    import concourse.tile as tile
    from concourse import mybir

    dt = mybir.dt
    nc = bacc.Bacc("TRN2", target_bir_lowering=False, debug=False,
                   enable_asserts=False)

    xg = nc.dram_tensor("xg", (NCH, 128, 2, 2048), dt.int8, kind="ExternalInput")
    wq = nc.dram_tensor("wq", (EL, 4, 128, 512), dt.int8, kind="ExternalInput")
    gates = nc.dram_tensor("gates", (128, 512), dt.float32, kind="ExternalInput")
    srcdst = nc.dram_tensor("srcdst", (128, 512), dt.int32, kind="ExternalInput")
    xsq = nc.dram_tensor("xsq", (128, 512), dt.bfloat16, kind="ExternalInput")
    # chunk-0 fast path: same data pre-laid-out at the row-tile partitions
    # (partition 32r+k = [kb=k, half r%2 of chunk 0]); bytes:
    # [0:1024) gates bf16, [1024:3072) srcdst i32, [3072:4096) xsq bf16
    a2b = nc.dram_tensor("a2b", (128, 4096), dt.int8, kind="ExternalInput")
    # mw bytes: [0:512) msl bf16 (row-tile replicated), [512:640) wscol f32
    mw = nc.dram_tensor("mw", (128, 640), dt.int8, kind="ExternalInput")
    shared = nc.dram_tensor("shared", (TPC, N), dt.bfloat16, kind="ExternalInput")
    out = nc.dram_tensor("out", (TPC, N), dt.bfloat16, kind="ExternalOutput")
    xsgd = nc.dram_tensor("xsgd", (128, 512), dt.bfloat16, kind="Internal")

    AX = mybir.AxisListType
    OP = mybir.AluOpType

    with tile.TileContext(nc) as tc, ExitStack() as ctx:
        const = ctx.enter_context(tc.tile_pool(name="const", bufs=1))
        wraw_p = ctx.enter_context(tc.tile_pool(name="wraw", bufs=2))
        gat_p = ctx.enter_context(tc.tile_pool(name="gat", bufs=4))
        gt_p = ctx.enter_context(tc.tile_pool(name="gtp", bufs=4))
        xdq_p = ctx.enter_context(tc.tile_pool(name="xdq", bufs=3))
        sh_p = ctx.enter_context(tc.tile_pool(name="shp", bufs=3))
        ob_p = ctx.enter_context(tc.tile_pool(name="obp", bufs=3))
        sps_p = ctx.enter_context(tc.tile_pool(name="sps", bufs=1, space="PSUM"))
        ops_p = ctx.enter_context(tc.tile_pool(name="ops", bufs=4, space="PSUM"))

        # ---- phase A2: chunk-0 xsg, computed directly at row-tile partitions
        a2 = const.tile([128, 4096], dt.int8)
        nc.sync.dma_start(a2[:], a2b.ap())
        g2v = a2[:].bitcast(dt.bfloat16)[:, 0:512]
        s2v = a2[:].bitcast(dt.int32)[:, 256:768]
        x2v = a2[:].bitcast(dt.bfloat16)[:, 1536:2048]
        g32 = g2v.rearrange("p (t j) -> p t j", j=8)
        sums2 = const.tile([128, 64], dt.float32)
        nc.vector.tensor_reduce(sums2[:], g32, AX.X, OP.add)
        rec2 = const.tile([128, 64], dt.float32)
        nc.vector.reciprocal(rec2[:], sums2[:])
        m2 = const.tile([128, 512], dt.bfloat16)
        nc.vector.scalar_tensor_tensor(m2[:], s2v, -1, x2v,
                                       OP.not_equal, OP.mult)
        gn2 = const.tile([128, 512], dt.float32)
        nc.vector.scalar_tensor_tensor(
            gn2[:].rearrange("p (t j) -> p t j", j=8), g32, 1.0,
            rec2[:].unsqueeze(2).broadcast_to([128, 64, 8]), OP.mult, OP.mult)
        xsg2 = const.tile([128, 512], dt.bfloat16)
        nc.vector.tensor_tensor(xsg2[:], gn2[:], m2[:], OP.mult)

        # ---- phase A: xsg = normalized+masked gates * x-scales (kb-major)
        gsb = const.tile([128, 512], dt.float32)
        nc.sync.dma_start(gsb[:], gates.ap())
        ssb = const.tile([128, 512], dt.int32)
        nc.sync.dma_start(ssb[:], srcdst.ap())
        xsb = const.tile([128, 512], dt.bfloat16)
        nc.sync.dma_start(xsb[:], xsq.ap())
        mwt = const.tile([128, 640], dt.int8)
        nc.sync.dma_start(mwt[:], mw.ap())
        msl = mwt[:].bitcast(dt.bfloat16)[:, 0:256]
        wsc = mwt[:].bitcast(dt.float32)[:, 128:160]
        # layout: partition p = q*4 + k (q = chunk-half 0..31, k = kb),
        # so chunk (ch, h)'s 4 kb rows are partitions 4*(2ch+h) .. +4
        # Emitted after s_stage(0) so chunk 0's DVE chain runs first.
        def phase_a_main():
            g3 = gsb[:].rearrange("p (t j) -> p t j", j=8)
            sums = const.tile([128, 64], dt.float32)
            nc.vector.tensor_reduce(sums[:], g3, AX.X, OP.add)
            rec = const.tile([128, 64], dt.float32)
            nc.vector.reciprocal(rec[:], sums[:])
            gm = const.tile([128, 512], dt.bfloat16)
            nc.vector.scalar_tensor_tensor(gm[:], ssb[:], -1, xsb[:],
                                           OP.not_equal, OP.mult)
            gn = const.tile([128, 512], dt.float32)
            nc.vector.scalar_tensor_tensor(
                gn[:].rearrange("p (t j) -> p t j", j=8), g3, 1.0,
                rec[:].unsqueeze(2).broadcast_to([128, 64, 8]),
                OP.mult, OP.mult)
            xsgall = const.tile([128, 512], dt.bfloat16)
            nc.vector.tensor_tensor(xsgall[:], gn[:], gm[:], OP.mult)
            nc.sync.dma_start(xsgd.ap(), xsgall[:])

        wdeq_t = [const.tile([128, 4 * 512], dt.bfloat16, name=f"wdeq{e}")
                  for e in range(EL)]

        def phase_b(e, gs=(0, 1, 2, 3)):
            wdv = wdeq_t[e][:].rearrange("p (g n) -> p g n", g=4)
            for g in gs:
                wr = wraw_p.tile([128, 512], dt.int8)
                nc.sync.dma_start(wr[:], wq.ap()[e, g])
                for nb in range(4):
                    col = e * 8 + (g // 2) * 4 + nb
                    nc.scalar.mul(wdv[:, g, nb * 128:(nb + 1) * 128],
                                  wr[:, nb * 128:(nb + 1) * 128],
                                  wsc[:, col:col + 1])

        phase_b(0, (0,))

        # ---- software-pipelined main loop: S-stage runs LA chunks ahead
        LA = 2
        xdqs = {}

        def s_stage(ch):
            if ch == 0:
                mv = xsg2[:]
            else:
                gt = gt_p.tile([128, 512], dt.bfloat16)
                for r in range(4):
                    q0 = 4 * (ch * 2 + r % 2)
                    nc.sync.dma_start(gt[32 * r:32 * r + 4, :],
                                      xsgd.ap()[q0:q0 + 4, :])
                mv = gt[:]
            Xg = gat_p.tile([128, 2, 2048], dt.int8)
            nc.sync.dma_start(Xg[:], xg.ap()[ch])
            sp = [sps_p.tile([128, 512], dt.float32, name=f"sp{r}")
                  for r in range(4)]
            for r in range(4):
                c = r // 2
                nc.tensor.matmul(
                    sp[r][:],
                    msl[32 * r:32 * r + 4, c * 128:(c + 1) * 128],
                    mv[32 * r:32 * r + 4, :],
                    start=True, stop=True, tile_position=(32 * r, 0))
            xdq = xdq_p.tile([128, 2, 2048], dt.bfloat16)
            for r in range(4):
                c, h = r // 2, r % 2
                nc.vector.scalar_tensor_tensor(
                    xdq[:, c, h * 1024:(h + 1) * 1024]
                        .rearrange("p (i b) -> p i b", b=2),
                    Xg[:, c, h * 1024:(h + 1) * 1024]
                        .rearrange("p (i b) -> p i b", b=2), 1.0,
                    sp[r][:].unsqueeze(2).broadcast_to([128, 512, 2]),
                    OP.mult, OP.mult)
            return xdq

        def main_stage(ch, xdq):
            e = ch // (4096 // R)
            ops = ops_p.tile([128, 512], dt.float32)
            wdv = wdeq_t[e][:].rearrange("p (g n) -> p g n", g=4)
            xv = xdq[:].rearrange("p c (t j b) -> p c b j t", t=128, j=8)
            first = True
            for c in range(2):
                for b in range(2):
                    g = 2 * c + b
                    for j in range(8):
                        nc.tensor.matmul(ops[:], xv[:, c, b, j, :],
                                         wdv[:, g, :],
                                         start=first,
                                         stop=(c == 1 and b == 1 and j == 7))
                        first = False
            sh = sh_p.tile([128, 512], dt.bfloat16)
            nc.sync.dma_start(sh[:], shared.ap()[ch * 128:(ch + 1) * 128, :])
            ob = ob_p.tile([128, 512], dt.bfloat16)
            nc.vector.scalar_tensor_tensor(ob[:], ops[:], 1.0, sh[:],
                                           OP.mult, OP.add)
            nc.sync.dma_start(out.ap()[ch * 128:(ch + 1) * 128, :], ob[:])

        for ch in range(n_chunks + LA):
            if ch == 1:
                phase_b(0, (1, 2, 3))
            if ch >= 4 and ch % 4 == 0 and ch // 4 < EL:
                phase_b(ch // 4)
            if ch < n_chunks:
                xdqs[ch] = s_stage(ch)
            if ch == 0:
                phase_a_main()
            if ch >= LA:
                main_stage(ch - LA, xdqs.pop(ch - LA))

    nc.compile()
    return nc


def _prep_inputs(input, weight, top_k_gates, token_indices, src_to_dst,
                 token_count, shared_output, weight_scale, input_scale):
    bf16 = ml_dtypes.bfloat16
    x = np.ascontiguousarray(np.asarray(input, dtype=np.int8))
    w = np.asarray(weight, dtype=np.int8)
    tkg = np.asarray(top_k_gates, dtype=np.float32)
    ti = np.asarray(token_indices, dtype=np.int32)
    s2d = np.asarray(src_to_dst, dtype=np.int32)
    sho = np.asarray(shared_output).astype(bf16)
    wsc = np.asarray(weight_scale, dtype=np.float32)
    xsc = np.asarray(input_scale, dtype=np.float32)

    p = np.arange(128)
    g = np.arange(4)
    kperm = 256 * (g[:, None] // 2) + 2 * p[None, :] + (g[:, None] % 2)  # [4,128]

    mselh = np.zeros((4, 256), bf16)
    for c in range(2):
        for pp in range(128):
            mselh[2 * c + pp // 64, c * 128 + pp] = 1.0
    mselrep = np.zeros((128, 256), bf16)
    for r in range(4):
        mselrep[32 * r:32 * r + 4, :] = mselh

    in_maps = []
    for cid in range(NCORES):
        e0 = cid * EL
        t0 = cid * TPC
        tl = ti[cid * RPC:(cid + 1) * RPC]
        # dispatch: pack this core's routed rows, pre-transposed
        xr = x[tl]                                   # [RPC, 512]
        arr = xr.reshape(NCH, R, 2, 128, 2)          # [ch, i, c, p, b]
        xgh = np.ascontiguousarray(
            np.transpose(arr, (0, 3, 2, 1, 4))).reshape(NCH, 128, 2, 2048)
        # q-major layout: partition p = q*4 + k holds [kb=k, rows q*512:+512)
        xs_rows = xsc[tl].astype(bf16)               # [RPC, 4]
        xsqh = np.ascontiguousarray(
            xs_rows.reshape(32, 512, 4).transpose(0, 2, 1)).reshape(128, 512)
        gfl = tkg[t0:t0 + TPC].reshape(-1)
        gtsh = np.ascontiguousarray(np.broadcast_to(
            gfl.reshape(32, 1, 512), (32, 4, 512))).reshape(128, 512)
        sfl = s2d[t0:t0 + TPC].reshape(-1)
        ssbh = np.ascontiguousarray(np.broadcast_to(
            sfl.reshape(32, 1, 512), (32, 4, 512))).reshape(128, 512)
        wq_h = np.ascontiguousarray(w[e0:e0 + EL][:, kperm, :])  # [EL,4,128,512]
        wcol = np.zeros((128, EL * 8), np.float32)
        for e in range(EL):
            for c in range(2):
                for nb in range(4):
                    wcol[:, e * 8 + c * 4 + nb] = wsc[e0 + e, 2 * c + p // 64, nb]
        mwb = np.zeros((128, 640), np.int8)
        mwb[:, 0:512] = mselrep.view(np.int8).reshape(128, 512)
        mwb[:, 512:640] = wcol.view(np.int8).reshape(128, 128)
        # chunk-0 fast-path blob (padding partitions get gates=1 so the
        # normalization reciprocal stays finite; they are never read)
        g2 = np.ones((128, 512), bf16)
        s2 = np.full((128, 512), -1, np.int32)
        x2 = np.zeros((128, 512), bf16)
        for r in range(4):
            h = r % 2
            for k in range(4):
                g2[32 * r + k] = gfl[h * 512:(h + 1) * 512].astype(bf16)
                s2[32 * r + k] = sfl[h * 512:(h + 1) * 512]
                x2[32 * r + k] = xs_rows[h * 512:(h + 1) * 512, k]
        a2h = np.zeros((128, 4096), np.int8)
        a2h[:, 0:1024] = g2.view(np.int8).reshape(128, 1024)
        a2h[:, 1024:3072] = s2.view(np.int8).reshape(128, 2048)
        a2h[:, 3072:4096] = x2.view(np.int8).reshape(128, 1024)
        in_maps.append({
            "xg": xgh,
            "wq": wq_h,
            "gates": gtsh,
            "srcdst": ssbh,
            "xsq": xsqh,
            "mw": mwb,
            "a2b": a2h,
            "shared": np.ascontiguousarray(sho[t0:t0 + TPC]),
        })
    return in_maps


def kernel(**inputs):
    from concourse import bass_utils
    if "nc" not in _cache:
        _cache["nc"] = _build()
    nc = _cache["nc"]
    in_maps = _prep_inputs(**inputs)
    import os
    res = bass_utils.run_bass_kernel_spmd(
        nc, in_maps, core_ids=list(range(NCORES)),
        trace=os.environ.get("BASS_TRACE") == "1")
    _cache["last_results"] = res
    out = np.concatenate([res.results[c]["out"] for c in range(NCORES)], axis=0)
    return out


# revision 62
# speedup vs baseline: 1.0095x; 1.0095x over previous
"""MoE grouped w8a8 block-quant GEMM + gated combine for 8 Trainium2 cores.

Sharding (expert-parallel, per the hint): core c owns experts [4c,4c+4),
their routed rows [c*16384,(c+1)*16384) (uniform token_count=4096), and
tokens [c*2048,(c+1)*2048). Routed rows are dispatched to their owning
core on the host (the all-to-all dispatch step): x rows are packed
pre-transposed into a K-on-partitions layout, so the device reads them
with fat sequential DMAs. All arithmetic (gate normalization/masking,
scale products, dequant, GEMM, combine) runs on device.

Device pipeline per core:
  Phase A (once): normalize+mask gates and multiply by x-scales in a
    q-major [128, 512] layout (partition q*4+k = kb k, rows q*512..) ->
    xsg table written to DRAM; a duplicate mini phase-A computes chunk
    0's xsg directly at the row-tile partitions (no DRAM roundtrip).
  Phase B: dequant int8 weights -> bf16 on the scalar engine, one
    expert ahead of use, interleaved into the loop.
  Main loop, software-pipelined (S-stage runs LA=2 chunks ahead of the
  GEMM stage), per chunk of R=1024 routed rows (= 128 tokens):
    1. Sequential DMA pulls the pre-dispatched x chunk [128, 2, 2048]
       int8; 4 small DMAs stage this chunk's xsg rows at partitions
       32r..32r+3.
    2. Four concurrent row-tiled K=4 matmuls (tile_position=(32r,0))
       broadcast xsg to 128 partitions (S[p] = xsg[kb(p)]); the scalar
       engine evicts the four 1-bank PSUM tiles to a bf16 S tile.
    3. Two DVE ops dequantize: xdq = int8 * S (bf16), gates folded in.
    4. 32 back-to-back matmuls accumulate the gated expert GEMM into
       [128 tokens, 512] PSUM (top-k combine == PSUM accumulation).
    5. DVE adds shared_output, writes bf16; DMA out.
"""

import numpy as np
import ml_dtypes

T, TOPK, K, N, E, B = 16384, 8, 512, 512, 32, 128
ROWS = T * TOPK
NCORES = 8
EL = E // NCORES            # experts per core
RPC = ROWS // NCORES        # routed rows per core
TPC = T // NCORES           # tokens per core
R = 1024                    # rows per chunk
NCH = RPC // R              # chunks per core

_cache = {}


def _build(n_chunks=NCH):
    from contextlib import ExitStack
    import concourse.bass as bass
    import concourse.bacc as bacc
    import concourse.tile as tile
    from concourse import mybir

    dt = mybir.dt
    nc = bacc.Bacc("TRN2", target_bir_lowering=False, debug=False,
                   enable_asserts=False)

    xg = nc.dram_tensor("xg", (NCH, 128, 2, 2048), dt.int8, kind="ExternalInput")
    wq = nc.dram_tensor("wq", (EL, 4, 128, 512), dt.int8, kind="ExternalInput")
    gates = nc.dram_tensor("gates", (128, 512), dt.float32, kind="ExternalInput")
    srcdst = nc.dram_tensor("srcdst", (128, 512), dt.int32, kind="ExternalInput")
    xsq = nc.dram_tensor("xsq", (128, 512), dt.bfloat16, kind="ExternalInput")
    # chunk-0 fast path: same data pre-laid-out at the row-tile partitions
    # (partition 32r+k = [kb=k, half r%2 of chunk 0]); gates ship first
    # (the reduce only needs them); a2s = srcdst i32 + xsq bf16
    a2g = nc.dram_tensor("a2g", (128, 1024), dt.int8, kind="ExternalInput")
    a2s = nc.dram_tensor("a2s", (128, 3072), dt.int8, kind="ExternalInput")
    # mw bytes: [0:512) msl bf16 (row-tile replicated), [512:640) wscol f32
    mw = nc.dram_tensor("mw", (128, 640), dt.int8, kind="ExternalInput")
    shared = nc.dram_tensor("shared", (TPC, N), dt.bfloat16, kind="ExternalInput")
    out = nc.dram_tensor("out", (TPC, N), dt.bfloat16, kind="ExternalOutput")
    xsgd = nc.dram_tensor("xsgd", (128, 512), dt.bfloat16, kind="Internal")

    AX = mybir.AxisListType
    OP = mybir.AluOpType

    with tile.TileContext(nc) as tc, ExitStack() as ctx:
        const = ctx.enter_context(tc.tile_pool(name="const", bufs=1))
        wraw_p = ctx.enter_context(tc.tile_pool(name="wraw", bufs=2))
        gat_p = ctx.enter_context(tc.tile_pool(name="gat", bufs=4))
        gt_p = ctx.enter_context(tc.tile_pool(name="gtp", bufs=4))
        xdq_p = ctx.enter_context(tc.tile_pool(name="xdq", bufs=3))
        sh_p = ctx.enter_context(tc.tile_pool(name="shp", bufs=3))
        ob_p = ctx.enter_context(tc.tile_pool(name="obp", bufs=3))
        sps_p = ctx.enter_context(tc.tile_pool(name="sps", bufs=1, space="PSUM"))
        ops_p = ctx.enter_context(tc.tile_pool(name="ops", bufs=4, space="PSUM"))

        # ---- phase A2: chunk-0 xsg, computed directly at row-tile partitions
        a2 = const.tile([128, 1024], dt.int8)
        nc.sync.dma_start(a2[:], a2g.ap())
        a2x = const.tile([128, 3072], dt.int8)
        nc.sync.dma_start(a2x[:], a2s.ap())
        g2v = a2[:].bitcast(dt.bfloat16)[:, 0:512]
        s2v = a2x[:].bitcast(dt.int32)[:, 0:512]
        x2v = a2x[:].bitcast(dt.bfloat16)[:, 1024:1536]
        g32 = g2v.rearrange("p (t j) -> p t j", j=8)
        sums2 = const.tile([128, 64], dt.float32)
        nc.vector.tensor_reduce(sums2[:], g32, AX.X, OP.add)
        rec2 = const.tile([128, 64], dt.float32)
        nc.vector.reciprocal(rec2[:], sums2[:])
        m2 = const.tile([128, 512], dt.bfloat16)
        nc.vector.scalar_tensor_tensor(m2[:], s2v, -1, x2v,
                                       OP.not_equal, OP.mult)
        gn2 = const.tile([128, 512], dt.float32)
        nc.vector.scalar_tensor_tensor(
            gn2[:].rearrange("p (t j) -> p t j", j=8), g32, 1.0,
            rec2[:].unsqueeze(2).broadcast_to([128, 64, 8]), OP.mult, OP.mult)
        xsg2 = const.tile([128, 512], dt.bfloat16)
        nc.vector.tensor_tensor(xsg2[:], gn2[:], m2[:], OP.mult)

        # ---- phase A: xsg = normalized+masked gates * x-scales (kb-major)
        gsb = const.tile([128, 512], dt.float32)
        nc.sync.dma_start(gsb[:], gates.ap())
        ssb = const.tile([128, 512], dt.int32)
        nc.sync.dma_start(ssb[:], srcdst.ap())
        xsb = const.tile([128, 512], dt.bfloat16)
        nc.sync.dma_start(xsb[:], xsq.ap())
        mwt = const.tile([128, 640], dt.int8)
        nc.sync.dma_start(mwt[:], mw.ap())
        msl = mwt[:].bitcast(dt.bfloat16)[:, 0:256]
        wsc = mwt[:].bitcast(dt.float32)[:, 128:160]
        # layout: partition p = q*4 + k (q = chunk-half 0..31, k = kb),
        # so chunk (ch, h)'s 4 kb rows are partitions 4*(2ch+h) .. +4
        # Emitted after s_stage(0) so chunk 0's DVE chain runs first.
        def phase_a_main():
            g3 = gsb[:].rearrange("p (t j) -> p t j", j=8)
            sums = const.tile([128, 64], dt.float32)
            nc.vector.tensor_reduce(sums[:], g3, AX.X, OP.add)
            rec = const.tile([128, 64], dt.float32)
            nc.vector.reciprocal(rec[:], sums[:])
            gm = const.tile([128, 512], dt.bfloat16)
            nc.vector.scalar_tensor_tensor(gm[:], ssb[:], -1, xsb[:],
                                           OP.not_equal, OP.mult)
            gn = const.tile([128, 512], dt.float32)
            nc.vector.scalar_tensor_tensor(
                gn[:].rearrange("p (t j) -> p t j", j=8), g3, 1.0,
                rec[:].unsqueeze(2).broadcast_to([128, 64, 8]),
                OP.mult, OP.mult)
            xsgall = const.tile([128, 512], dt.bfloat16)
            nc.vector.tensor_tensor(xsgall[:], gn[:], gm[:], OP.mult)
            nc.sync.dma_start(xsgd.ap(), xsgall[:])

        wdeq_t = [const.tile([128, 4 * 512], dt.bfloat16, name=f"wdeq{e}")
                  for e in range(EL)]

        def phase_b(e, gs=(0, 1, 2, 3)):
            wdv = wdeq_t[e][:].rearrange("p (g n) -> p g n", g=4)
            for g in gs:
                wr = wraw_p.tile([128, 512], dt.int8)
                nc.sync.dma_start(wr[:], wq.ap()[e, g])
                for nb in range(4):
                    col = e * 8 + (g // 2) * 4 + nb
                    nc.scalar.mul(wdv[:, g, nb * 128:(nb + 1) * 128],
                                  wr[:, nb * 128:(nb + 1) * 128],
                                  wsc[:, col:col + 1])

        phase_b(0, (0,))

        # ---- software-pipelined main loop: S-stage runs LA chunks ahead
        LA = 2
        xdqs = {}

        def s_stage(ch):
            if ch == 0:
                mv = xsg2[:]
            else:
                gt = gt_p.tile([128, 512], dt.bfloat16)
                for r in range(4):
                    q0 = 4 * (ch * 2 + r % 2)
                    nc.sync.dma_start(gt[32 * r:32 * r + 4, :],
                                      xsgd.ap()[q0:q0 + 4, :])
                mv = gt[:]
            Xg = gat_p.tile([128, 2, 2048], dt.int8)
            nc.sync.dma_start(Xg[:], xg.ap()[ch])
            sp = [sps_p.tile([128, 512], dt.float32, name=f"sp{r}")
                  for r in range(4)]
            for r in range(4):
                c = r // 2
                nc.tensor.matmul(
                    sp[r][:],
                    msl[32 * r:32 * r + 4, c * 128:(c + 1) * 128],
                    mv[32 * r:32 * r + 4, :],
                    start=True, stop=True, tile_position=(32 * r, 0))
            xdq = xdq_p.tile([128, 2, 2048], dt.bfloat16)
            for r in range(4):
                c, h = r // 2, r % 2
                nc.vector.scalar_tensor_tensor(
                    xdq[:, c, h * 1024:(h + 1) * 1024]
                        .rearrange("p (i b) -> p i b", b=2),
                    Xg[:, c, h * 1024:(h + 1) * 1024]
                        .rearrange("p (i b) -> p i b", b=2), 1.0,
                    sp[r][:].unsqueeze(2).broadcast_to([128, 512, 2]),
                    OP.mult, OP.mult)
            return xdq

        def main_stage(ch, xdq):
            e = ch // (4096 // R)
            ops = ops_p.tile([128, 512], dt.float32)
            wdv = wdeq_t[e][:].rearrange("p (g n) -> p g n", g=4)
            xv = xdq[:].rearrange("p c (t j b) -> p c b j t", t=128, j=8)
            first = True
            for c in range(2):
                for b in range(2):
                    g = 2 * c + b
                    for j in range(8):
                        nc.tensor.matmul(ops[:], xv[:, c, b, j, :],
                                         wdv[:, g, :],
                                         start=first,
                                         stop=(c == 1 and b == 1 and j == 7))
                        first = False
            sh = sh_p.tile([128, 512], dt.bfloat16)
            nc.sync.dma_start(sh[:], shared.ap()[ch * 128:(ch + 1) * 128, :])
            ob = ob_p.tile([128, 512], dt.bfloat16)
            nc.vector.scalar_tensor_tensor(ob[:], ops[:], 1.0, sh[:],
                                           OP.mult, OP.add)
            nc.sync.dma_start(out.ap()[ch * 128:(ch + 1) * 128, :], ob[:])

        for ch in range(n_chunks + LA):
            if ch == 1:
                phase_b(0, (1, 2, 3))
            if ch >= 4 and ch % 4 == 0 and ch // 4 < EL:
                phase_b(ch // 4)
            if ch < n_chunks:
                xdqs[ch] = s_stage(ch)
            if ch == 0:
                phase_a_main()
            if ch >= LA:
                main_stage(ch - LA, xdqs.pop(ch - LA))

    nc.compile()
    return nc


def _prep_inputs(input, weight, top_k_gates, token_indices, src_to_dst,
                 token_count, shared_output, weight_scale, input_scale):
    bf16 = ml_dtypes.bfloat16
    x = np.ascontiguousarray(np.asarray(input, dtype=np.int8))
    w = np.asarray(weight, dtype=np.int8)
    tkg = np.asarray(top_k_gates, dtype=np.float32)
    ti = np.asarray(token_indices, dtype=np.int32)
    s2d = np.asarray(src_to_dst, dtype=np.int32)
    sho = np.asarray(shared_output).astype(bf16)
    wsc = np.asarray(weight_scale, dtype=np.float32)
    xsc = np.asarray(input_scale, dtype=np.float32)

    p = np.arange(128)
    g = np.arange(4)
    kperm = 256 * (g[:, None] // 2) + 2 * p[None, :] + (g[:, None] % 2)  # [4,128]

    mselh = np.zeros((4, 256), bf16)
    for c in range(2):
        for pp in range(128):
            mselh[2 * c + pp // 64, c * 128 + pp] = 1.0
    mselrep = np.zeros((128, 256), bf16)
    for r in range(4):
        mselrep[32 * r:32 * r + 4, :] = mselh

    in_maps = []
    for cid in range(NCORES):
        e0 = cid * EL
        t0 = cid * TPC
        tl = ti[cid * RPC:(cid + 1) * RPC]
        # dispatch: pack this core's routed rows, pre-transposed
        xr = x[tl]                                   # [RPC, 512]
        arr = xr.reshape(NCH, R, 2, 128, 2)          # [ch, i, c, p, b]
        xgh = np.ascontiguousarray(
            np.transpose(arr, (0, 3, 2, 1, 4))).reshape(NCH, 128, 2, 2048)
        # q-major layout: partition p = q*4 + k holds [kb=k, rows q*512:+512)
        xs_rows = xsc[tl].astype(bf16)               # [RPC, 4]
        xsqh = np.ascontiguousarray(
            xs_rows.reshape(32, 512, 4).transpose(0, 2, 1)).reshape(128, 512)
        gfl = tkg[t0:t0 + TPC].reshape(-1)
        gtsh = np.ascontiguousarray(np.broadcast_to(
            gfl.reshape(32, 1, 512), (32, 4, 512))).reshape(128, 512)
        sfl = s2d[t0:t0 + TPC].reshape(-1)
        ssbh = np.ascontiguousarray(np.broadcast_to(
            sfl.reshape(32, 1, 512), (32, 4, 512))).reshape(128, 512)
        wq_h = np.ascontiguousarray(w[e0:e0 + EL][:, kperm, :])  # [EL,4,128,512]
        wcol = np.zeros((128, EL * 8), np.float32)
        for e in range(EL):
            for c in range(2):
                for nb in range(4):
                    wcol[:, e * 8 + c * 4 + nb] = wsc[e0 + e, 2 * c + p // 64, nb]
        mwb = np.zeros((128, 640), np.int8)
        mwb[:, 0:512] = mselrep.view(np.int8).reshape(128, 512)
        mwb[:, 512:640] = wcol.view(np.int8).reshape(128, 128)
        # chunk-0 fast-path blob (padding partitions get gates=1 so the
        # normalization reciprocal stays finite; they are never read)
        g2 = np.ones((128, 512), bf16)
        s2 = np.full((128, 512), -1, np.int32)
        x2 = np.zeros((128, 512), bf16)
        for r in range(4):
            h = r % 2
            for k in range(4):
                g2[32 * r + k] = gfl[h * 512:(h + 1) * 512].astype(bf16)
                s2[32 * r + k] = sfl[h * 512:(h + 1) * 512]
                x2[32 * r + k] = xs_rows[h * 512:(h + 1) * 512, k]
        a2gh = np.ascontiguousarray(g2.view(np.int8).reshape(128, 1024))
        a2sh = np.zeros((128, 3072), np.int8)
        a2sh[:, 0:2048] = s2.view(np.int8).reshape(128, 2048)
        a2sh[:, 2048:3072] = x2.view(np.int8).reshape(128, 1024)
        in_maps.append({
            "xg": xgh,
            "wq": wq_h,
            "gates": gtsh,
            "srcdst": ssbh,
            "xsq": xsqh,
            "mw": mwb,
            "a2g": a2gh,
            "a2s": a2sh,
            "shared": np.ascontiguousarray(sho[t0:t0 + TPC]),
        })
    return in_maps


def kernel(**inputs):
    from concourse import bass_utils
    if "nc" not in _cache:
        _cache["nc"] = _build()
    nc = _cache["nc"]
    in_maps = _prep_inputs(**inputs)
    import os
    res = bass_utils.run_bass_kernel_spmd(
        nc, in_maps, core_ids=list(range(NCORES)),
        trace=os.environ.get("BASS_TRACE") == "1")
    _cache["last_results"] = res
    out = np.concatenate([res.results[c]["out"] for c in range(NCORES)], axis=0)
    return out
